# revision 1
# baseline (speedup 1.0000x reference)
"""Expert-parallel MoE FFN kernel for Trainium2 (8 NeuronCores, 1 expert/core).

Computes, per expert e:
    out_e = gelu(x_e @ w1_e + b1_e) @ w2_e + b2_e
with x [B, E*CAP, D] sharded on the expert dim: x_e = x[:, e*CAP:(e+1)*CAP, :]
flattened to [B*CAP, D].

Per-core kernel layout (all SBUF tiles are [128 partitions, free]):
  - host pre-packs x_e (transposed), w1_e, w2_e into partition-major tiled
    layouts so every streamed SBUF strip is one contiguous ~16KB descriptor
    per partition (max DMA efficiency), in bf16.
  - mm1: psum_hT[f,m] += w1_tile[k,f].T @ xT_tile[k,m]  (16 k-tiles)
          evict: G = Gelu(psum + b1) on ScalarE (per-partition bias), bf16
  - mm2: psum_oT[d,m] += w2_tile[f,d].T @ G[f,m]        (64 f-tiles)
          evict: oT = psum + b2 (DVE broadcast add), fp32, DMA to DRAM
  - output returned transposed (oT [D, M]) and un-transposed on host.
"""

import os
import sys

import numpy as np

for _p in ("/opt/trn_rl_repo", os.path.expanduser("~/.axon_site/_ro/trn_rl_repo")):
    if os.path.isdir(_p) and _p not in sys.path:
        sys.path.insert(0, _p)

import ml_dtypes

BF16 = ml_dtypes.bfloat16

# Problem shapes (hardcoded per contest contract)
E, B, CAP, D, F = 8, 4, 1024, 2048, 8192
M = B * CAP  # tokens per expert
P = 128

# Tiling
MB = 512              # m-block (columns of xT/G, rows of out) == matmul moving dim
N_MB = M // MB        # 8
KB = D // P           # 16 k sub-tiles
FB = F // P           # 64 f blocks
FS = 4                # f blocks per w1 streaming chunk (chunk width 512)
N_FS = FB // FS       # 16
DB = D // P           # 16 d blocks

TRACE = False          # set True (module-level) to capture an NTFF profile
LAST_EXEC_NS = None    # filled when TRACE and profiling succeeded
LAST_RESULTS = None


def _pack_xt(xT, n_mb, mb, kb):
    """xT [K, M] -> [P, n_mb, kb, mb] with [p, i, ko, mi] = xT[ko*P+p, i*mb+mi]."""
    k, m = xT.shape
    return np.ascontiguousarray(
        xT.reshape(kb, P, n_mb, mb).transpose(1, 2, 0, 3)
    )


def _pack_w1(w1, n_fs, fsw, kb):
    """w1 [K, F] -> [P, n_fs, kb, fsw] with [p, s, ko, fi] = w1[ko*P+p, s*fsw+fi]."""
    return np.ascontiguousarray(
        w1.reshape(kb, P, n_fs, fsw).transpose(1, 2, 0, 3)
    )


def _pack_w2(w2, db, fb):
    """w2 [F, D] -> [P, db, fb, P] with [p, j, fo, ds] = w2[fo*P+p, j*P+ds]."""
    return np.ascontiguousarray(
        w2.reshape(fb, P, db, P).transpose(1, 2, 0, 3)
    )


def _build_nc(dims=None, act="Gelu"):
    """Build the per-core Bass program. dims overrides (D,F,M,MB) for testing."""
    import concourse.bacc as bacc
    import concourse.mybir as mybir
    import concourse.tile as tile

    d_, f_, m_, mb_ = dims if dims is not None else (D, F, M, MB)
    kb_ = d_ // P
    fb_ = f_ // P
    fs_ = min(FS, fb_)
    n_fs_ = fb_ // fs_
    fsw_ = fs_ * P
    db_ = d_ // P
    n_mb_ = m_ // mb_

    nc = bacc.Bacc(None, target_bir_lowering=False)

    xt_h = nc.dram_tensor(
        "xt", [P, n_mb_, kb_, mb_], mybir.dt.bfloat16, kind="ExternalInput"
    )
    w1_h = nc.dram_tensor(
        "w1", [P, n_fs_, kb_, fsw_], mybir.dt.bfloat16, kind="ExternalInput"
    )
    w2_h = nc.dram_tensor(
        "w2", [P, db_, fb_, P], mybir.dt.bfloat16, kind="ExternalInput"
    )
    b1_h = nc.dram_tensor("b1", [P, fb_], mybir.dt.float32, kind="ExternalInput")
    b2_h = nc.dram_tensor("b2", [P, db_], mybir.dt.float32, kind="ExternalInput")
    ot_h = nc.dram_tensor("ot", [d_, m_], mybir.dt.float32, kind="ExternalOutput")

    ot_r = ot_h[:, :].rearrange("(do p) m -> p do m", p=P)

    GELU = getattr(mybir.ActivationFunctionType, act)
    f32 = mybir.dt.float32
    bf16 = mybir.dt.bfloat16

    with tile.TileContext(nc) as tc:
        with (
            tc.tile_pool(name="const", bufs=1) as constp,
            tc.tile_pool(name="xt", bufs=2) as xtp,
            tc.tile_pool(name="w1", bufs=2) as w1p,
            tc.tile_pool(name="w2", bufs=2) as w2p,
            tc.tile_pool(name="g", bufs=fb_ + 16) as gp,
            tc.tile_pool(name="out", bufs=4) as outp,
            tc.tile_pool(name="ps1", bufs=3, space="PSUM") as ps1p,
            tc.tile_pool(name="ps2", bufs=3, space="PSUM") as ps2p,
        ):
            b1_sb = constp.tile([P, fb_], f32, tag="b1")
            b2_sb = constp.tile([P, db_], f32, tag="b2")
            nsplit = 8 if kb_ % 8 == 0 else (4 if kb_ % 4 == 0 else 1)

            for mb in range(n_mb_):
                m0 = mb * mb_
                xt_t = xtp.tile([P, kb_, mb_], bf16, tag="xt")
                if mb == 0 and nsplit > 1:
                    # Split the first x-block load (interleaved with the
                    # first w1 chunk below) so the first matmul group can
                    # start as soon as the earliest k-strips land; biases
                    # go on the queue right after the first pair.
                    q = kb_ // nsplit
                    w1_t0 = w1p.tile([P, kb_, fsw_], bf16, tag="w1")
                    for i in range(nsplit):
                        nc.sync.dma_start(
                            xt_t[:, i * q : (i + 1) * q, :],
                            xt_h[:, mb, i * q : (i + 1) * q],
                        )
                        nc.sync.dma_start(
                            w1_t0[:, i * q : (i + 1) * q, :],
                            w1_h[:, 0, i * q : (i + 1) * q],
                        )
                        if i == 0:
                            nc.sync.dma_start(b1_sb, b1_h[:, :])
                            nc.sync.dma_start(b2_sb, b2_h[:, :])
                else:
                    if mb == 0:
                        nc.sync.dma_start(b1_sb, b1_h[:, :])
                        nc.sync.dma_start(b2_sb, b2_h[:, :])
                    nc.sync.dma_start(xt_t, xt_h[:, mb])  # sync queue

                G = []
                for fs in range(n_fs_):
                    if mb == 0 and fs == 0 and nsplit > 1:
                        w1_t = w1_t0
                    else:
                        w1_t = w1p.tile([P, kb_, fsw_], bf16, tag="w1")
                        nc.sync.dma_start(w1_t, w1_h[:, fs])
                    for fb in range(fs_):
                        fi = fs * fs_ + fb
                        ps = ps1p.tile([P, mb_], f32, tag="ps1")
                        for k in range(kb_):
                            nc.tensor.matmul(
                                ps,
                                lhsT=w1_t[:, k, fb * P : (fb + 1) * P],
                                rhs=xt_t[:, k, :],
                                start=(k == 0),
                                stop=(k == kb_ - 1),
                            )
                        g = gp.tile([P, mb_], bf16, tag="g")
                        nc.scalar.activation(g, ps, GELU, bias=b1_sb[:, fi : fi + 1])
                        G.append(g)

                for db in range(db_):
                    w2_t = w2p.tile([P, fb_, P], bf16, tag="w2")
                    nc.sync.dma_start(w2_t, w2_h[:, db])
                    last_group = mb == n_mb_ - 1 and db == db_ - 1
                    if last_group and mb_ % 2 == 0:
                        # Split the final accumulation along m so the first
                        # half's eviction overlaps the second half's matmuls
                        # (nothing can hide the tail otherwise).
                        hm = mb_ // 2
                        for part in range(2):
                            msl = slice(part * hm, (part + 1) * hm)
                            ps2 = ps2p.tile([P, hm], f32, tag="ps2t", bufs=2)
                            for fi in range(fb_):
                                nc.tensor.matmul(
                                    ps2,
                                    lhsT=w2_t[:, fi, :],
                                    rhs=G[fi][:, msl],
                                    start=(fi == 0),
                                    stop=(fi == fb_ - 1),
                                )
                            o_t = outp.tile([P, hm], f32, tag="ot_tail", bufs=2)
                            nc.vector.tensor_add(
                                o_t,
                                ps2,
                                b2_sb[:, db : db + 1].to_broadcast([P, hm]),
                            )
                            nc.sync.dma_start(
                                ot_r[:, db, m0 + part * hm : m0 + (part + 1) * hm],
                                o_t,
                            )
                        continue
                    ps2 = ps2p.tile([P, mb_], f32, tag="ps2")
                    for fi in range(fb_):
                        nc.tensor.matmul(
                            ps2,
                            lhsT=w2_t[:, fi, :],
                            rhs=G[fi],
                            start=(fi == 0),
                            stop=(fi == fb_ - 1),
                        )
                    o_t = outp.tile([P, mb_], f32, tag="o")
                    nc.vector.tensor_add(
                        o_t, ps2, b2_sb[:, db : db + 1].to_broadcast([P, mb_])
                    )
                    nc.sync.dma_start(ot_r[:, db, m0 : m0 + mb_], o_t)

    nc.compile()
    return nc


_NC = None


def _get_nc():
    global _NC
    if _NC is None:
        _NC = _build_nc()
    return _NC


def _shard_inputs(x, w1, b1, w2, b2):
    x = np.asarray(x)
    w1, b1 = np.asarray(w1), np.asarray(b1)
    w2, b2 = np.asarray(w2), np.asarray(b2)
    in_maps = []
    for e in range(E):
        xeT = x[:, e * CAP : (e + 1) * CAP, :].reshape(M, D).T.astype(BF16)
        in_maps.append(
            {
                "xt": _pack_xt(xeT, N_MB, MB, KB),
                "w1": _pack_w1(w1[e].astype(BF16), N_FS, FS * P, KB),
                "w2": _pack_w2(w2[e].astype(BF16), DB, FB),
                "b1": np.ascontiguousarray(
                    b1[e].reshape(FB, P).T.astype(np.float32)
                ),
                "b2": np.ascontiguousarray(
                    b2[e].reshape(DB, P).T.astype(np.float32)
                ),
            }
        )
    return in_maps


def _ensure_axon_hooks():
    """bass_utils imports antenv.axon_hooks when tracing is requested (e.g.
    via BASS_TRACE); this image's antenv lacks that module. Provide it, and
    register the local NTFF hook when available, so tracing degrades
    gracefully instead of raising ImportError."""
    import types

    try:
        import antenv
    except Exception:
        return
    if getattr(antenv, "axon_hooks", None) is not None:
        return
    try:
        import antenv.axon_hooks  # noqa: F401

        return
    except Exception:
        pass
    mod = types.ModuleType("antenv.axon_hooks")
    _h = [None]
    mod.get_axon_ntff_profile_hook = lambda: _h[0]
    mod.set_axon_ntff_profile_hook = lambda h: _h.__setitem__(0, h)
    sys.modules["antenv.axon_hooks"] = mod
    antenv.axon_hooks = mod
    try:
        from trn_agent_boot.trn_boot import _ntff_profile_via_ctypes

        hk = _ntff_profile_via_ctypes("/opt/axon/libaxon_pjrt.so")
        if hk is not None:
            mod.set_axon_ntff_profile_hook(hk)
    except Exception:
        pass
    try:
        # artifact upload needs S3; fall back to the local dir on failure
        import concourse.bass_utils as bu

        _orig_upload = bu.upload_artifacts

        def _safe_upload(tmpdir):
            try:
                return _orig_upload(tmpdir)
            except Exception:
                return "local://" + str(tmpdir)

        bu.upload_artifacts = _safe_upload
    except Exception:
        pass


def _run_with_retry(nc, in_maps, trace, attempts=3):
    """The device intermittently fails with NRT_EXEC_UNIT_UNRECOVERABLE even
    on known-good kernels; retry after resetting the jax backend."""
    import time

    from concourse.bass_utils import run_bass_kernel_spmd

    last = None
    for attempt in range(attempts):
        try:
            return run_bass_kernel_spmd(
                nc,
                in_maps,
                core_ids=list(range(E)),
                trace=trace and attempt == 0,
            )
        except Exception as e:  # noqa: BLE001
            last = e
            if attempt == attempts - 1:
                break
            time.sleep(45 * (attempt + 1))
            try:
                import jax.extend.backend

                jax.extend.backend.clear_backends()
            except Exception:
                pass
    raise last


def kernel(x, w1, b1, w2, b2):
    global LAST_EXEC_NS, LAST_RESULTS
    _ensure_axon_hooks()

    nc = _get_nc()
    in_maps = _shard_inputs(x, w1, b1, w2, b2)
    res = _run_with_retry(nc, in_maps, TRACE)
    LAST_EXEC_NS = res.exec_time_ns
    LAST_RESULTS = res
    out = np.empty((B, E * CAP, D), dtype=np.float32)
    for e in range(E):
        out[:, e * CAP : (e + 1) * CAP, :] = (
            res.results[e]["ot"].T.reshape(B, CAP, D)
        )
    return out



# revision 2
# speedup vs baseline: 1.0504x; 1.0504x over previous
"""Expert-parallel MoE FFN kernel for Trainium2 (8 NeuronCores, 1 expert/core).

Computes, per expert e:
    out_e = gelu(x_e @ w1_e + b1_e) @ w2_e + b2_e
with x [B, E*CAP, D] sharded on the expert dim: x_e = x[:, e*CAP:(e+1)*CAP, :]
flattened to [B*CAP, D].

Per-core kernel layout (all SBUF tiles are [128 partitions, free]):
  - host pre-packs x_e (transposed), w1_e, w2_e into partition-major tiled
    layouts so every streamed SBUF strip is one contiguous ~16KB descriptor
    per partition (max DMA efficiency), in bf16.
  - mm1: psum_hT[f,m] += w1_tile[k,f].T @ xT_tile[k,m]  (16 k-tiles)
          evict: G = Gelu(psum + b1) on ScalarE (per-partition bias), bf16
          for the first FB_BF f-tiles, e4m3 for the last 2*P8 f-tiles.
  - mm2 mixed precision: the contraction over F=8192 is split: the first
    FB_BF*128 rows run as bf16 matmuls into psum_oT; the last 2*P8*128 rows
    run as fp8e4 DoubleRow matmuls (2 f-tiles per instruction, 2x rate)
    into a second psum, with w2 host-quantized at scale 2^10.  Eviction
    combines: oT = psum_bf16 + (psum_fp8 * 2^-10 + b2), fp32, DMA to DRAM.
    fp8 noise = 2.66%/operand over 14/64 of the contraction -> ~1.8e-2
    rel_l2 total, inside the 2e-2 gate; saves ~11% of mm2 matmul rows.
  - output returned transposed (oT [D, M]) and un-transposed on host.
"""

import os
import sys

import numpy as np

for _p in ("/opt/trn_rl_repo", os.path.expanduser("~/.axon_site/_ro/trn_rl_repo")):
    if os.path.isdir(_p) and _p not in sys.path:
        sys.path.insert(0, _p)

import ml_dtypes

BF16 = ml_dtypes.bfloat16
E4M3 = ml_dtypes.float8_e4m3  # TRN FP8_EXP4-compatible (max +-240)

# Problem shapes (hardcoded per contest contract)
E, B, CAP, D, F = 8, 4, 1024, 2048, 8192
M = B * CAP  # tokens per expert
P = 128

# Tiling
MB = 512              # m-block (columns of xT/G, rows of out) == matmul moving dim
N_MB = M // MB        # 8
KB = D // P           # 16 k sub-tiles
FB = F // P           # 64 f blocks
FS = 4                # f blocks per w1 streaming chunk (chunk width 512)
N_FS = FB // FS       # 16
DB = D // P           # 16 d blocks

# Mixed-precision split of mm2's contraction (FB = FB_BF + 2*P8)
P8 = 7                # fp8 DoubleRow pair-instructions per psum group
FB_BF = FB - 2 * P8   # bf16 f-tiles (50)
W8SCALE = 1024.0      # w2 fp8 quantization scale (power of two)

TRACE = False          # set True (module-level) to capture an NTFF profile
LAST_EXEC_NS = None    # filled when TRACE and profiling succeeded
LAST_RESULTS = None


def _pack_xt(xT, n_mb, mb, kb):
    """xT [K, M] -> [P, n_mb, kb, mb] with [p, i, ko, mi] = xT[ko*P+p, i*mb+mi]."""
    k, m = xT.shape
    return np.ascontiguousarray(
        xT.reshape(kb, P, n_mb, mb).transpose(1, 2, 0, 3)
    )


def _pack_w1(w1, n_fs, fsw, kb):
    """w1 [K, F] -> [P, n_fs, kb, fsw] with [p, s, ko, fi] = w1[ko*P+p, s*fsw+fi]."""
    return np.ascontiguousarray(
        w1.reshape(kb, P, n_fs, fsw).transpose(1, 2, 0, 3)
    )


def _pack_w2_bf(w2, db, fb_bf):
    """w2 rows [:fb_bf*P] -> [P, db, fb_bf, P], [p,j,fo,ds] = w2[fo*P+p, j*P+ds]."""
    return np.ascontiguousarray(
        w2[: fb_bf * P].reshape(fb_bf, P, db, P).transpose(1, 2, 0, 3)
    )


def _pack_w2_fp8(w2, db, fb_bf, p8):
    """w2 rows [fb_bf*P:] quantized e4m3 at W8SCALE -> [P, db, p8, 2, P] with
    [p, j, u, v, ds] = q(w2[(fb_bf+2u+v)*P+p, j*P+ds] * W8SCALE)."""
    tail = w2[fb_bf * P :]
    q = np.clip(tail * W8SCALE, -240.0, 240.0).astype(E4M3)
    return np.ascontiguousarray(
        q.reshape(p8, 2, P, db, P).transpose(2, 3, 0, 1, 4)
    )


def _build_nc(dims=None, act="Gelu"):
    """Build the per-core Bass program. dims overrides (D,F,M,MB) for testing."""
    import concourse.bacc as bacc
    import concourse.mybir as mybir
    import concourse.tile as tile

    d_, f_, m_, mb_ = dims if dims is not None else (D, F, M, MB)
    kb_ = d_ // P
    fb_ = f_ // P
    fs_ = min(FS, fb_)
    n_fs_ = fb_ // fs_
    fsw_ = fs_ * P
    db_ = d_ // P
    n_mb_ = m_ // mb_
    p8_ = P8
    fb_bf_ = fb_ - 2 * p8_

    nc = bacc.Bacc(None, target_bir_lowering=False)

    xt_h = nc.dram_tensor(
        "xt", [P, n_mb_, kb_, mb_], mybir.dt.bfloat16, kind="ExternalInput"
    )
    w1_h = nc.dram_tensor(
        "w1", [P, n_fs_, kb_, fsw_], mybir.dt.bfloat16, kind="ExternalInput"
    )
    w2_h = nc.dram_tensor(
        "w2", [P, db_, fb_bf_, P], mybir.dt.bfloat16, kind="ExternalInput"
    )
    w28_h = nc.dram_tensor(
        "w28", [P, db_, p8_, 2, P], mybir.dt.float8e4, kind="ExternalInput"
    )
    b1_h = nc.dram_tensor("b1", [P, fb_], mybir.dt.float32, kind="ExternalInput")
    b2_h = nc.dram_tensor("b2", [P, db_], mybir.dt.float32, kind="ExternalInput")
    ot_h = nc.dram_tensor("ot", [d_, m_], mybir.dt.float32, kind="ExternalOutput")

    ot_r = ot_h[:, :].rearrange("(do p) m -> p do m", p=P)

    GELU = getattr(mybir.ActivationFunctionType, act)
    IDENT = mybir.ActivationFunctionType.Identity
    DR = mybir.MatmulPerfMode.DoubleRow
    f32 = mybir.dt.float32
    bf16 = mybir.dt.bfloat16
    fp8 = mybir.dt.float8e4

    with tile.TileContext(nc) as tc:
        with (
            tc.tile_pool(name="const", bufs=1) as constp,
            tc.tile_pool(name="xt", bufs=2) as xtp,
            tc.tile_pool(name="w1", bufs=2) as w1p,
            tc.tile_pool(name="w2", bufs=2) as w2p,
            tc.tile_pool(name="w28", bufs=2) as w28p,
            tc.tile_pool(name="g", bufs=fb_bf_ + 12) as gp,
            tc.tile_pool(name="g8", bufs=2) as g8p,
            tc.tile_pool(name="out", bufs=4) as outp,
            tc.tile_pool(name="t8", bufs=4) as t8p,
            tc.tile_pool(name="ps1", bufs=3, space="PSUM") as ps1p,
            tc.tile_pool(name="ps2", bufs=2, space="PSUM") as ps2p,
            tc.tile_pool(name="ps8", bufs=2, space="PSUM") as ps8p,
        ):
            b1_sb = constp.tile([P, fb_], f32, tag="b1")
            b2_sb = constp.tile([P, db_], f32, tag="b2")
            nsplit = 8 if kb_ % 8 == 0 else (4 if kb_ % 4 == 0 else 1)

            for mb in range(n_mb_):
                m0 = mb * mb_
                xt_t = xtp.tile([P, kb_, mb_], bf16, tag="xt")
                if mb == 0 and nsplit > 1:
                    # Split the first x-block load (interleaved with the
                    # first w1 chunk below) so the first matmul group can
                    # start as soon as the earliest k-strips land; biases
                    # go on the queue right after the first pair.
                    q = kb_ // nsplit
                    w1_t0 = w1p.tile([P, kb_, fsw_], bf16, tag="w1")
                    for i in range(nsplit):
                        nc.sync.dma_start(
                            xt_t[:, i * q : (i + 1) * q, :],
                            xt_h[:, mb, i * q : (i + 1) * q],
                        )
                        nc.sync.dma_start(
                            w1_t0[:, i * q : (i + 1) * q, :],
                            w1_h[:, 0, i * q : (i + 1) * q],
                        )
                        if i == 0:
                            nc.sync.dma_start(b1_sb, b1_h[:, :])
                            nc.sync.dma_start(b2_sb, b2_h[:, :])
                else:
                    if mb == 0:
                        nc.sync.dma_start(b1_sb, b1_h[:, :])
                        nc.sync.dma_start(b2_sb, b2_h[:, :])
                    nc.sync.dma_start(xt_t, xt_h[:, mb])  # sync queue

                G = []
                g8_t = g8p.tile([P, 2 * p8_, mb_], fp8, tag="g8")
                for fs in range(n_fs_):
                    if mb == 0 and fs == 0 and nsplit > 1:
                        w1_t = w1_t0
                    else:
                        w1_t = w1p.tile([P, kb_, fsw_], bf16, tag="w1")
                        nc.sync.dma_start(w1_t, w1_h[:, fs])
                    for fb in range(fs_):
                        fi = fs * fs_ + fb
                        ps = ps1p.tile([P, mb_], f32, tag="ps1")
                        for k in range(kb_):
                            nc.tensor.matmul(
                                ps,
                                lhsT=w1_t[:, k, fb * P : (fb + 1) * P],
                                rhs=xt_t[:, k, :],
                                start=(k == 0),
                                stop=(k == kb_ - 1),
                            )
                        if fi < fb_bf_:
                            g = gp.tile([P, mb_], bf16, tag="g")
                            nc.scalar.activation(
                                g, ps, GELU, bias=b1_sb[:, fi : fi + 1]
                            )
                            G.append(g)
                        else:
                            nc.scalar.activation(
                                g8_t[:, fi - fb_bf_, :],
                                ps,
                                GELU,
                                bias=b1_sb[:, fi : fi + 1],
                            )

                for db in range(db_):
                    w2_t = w2p.tile([P, fb_bf_, P], bf16, tag="w2")
                    nc.sync.dma_start(w2_t, w2_h[:, db])
                    w28_t = w28p.tile([P, p8_, 2, P], fp8, tag="w28")
                    nc.sync.dma_start(w28_t, w28_h[:, db])

                    ps2 = ps2p.tile([P, mb_], f32, tag="ps2")
                    for fi in range(fb_bf_):
                        nc.tensor.matmul(
                            ps2,
                            lhsT=w2_t[:, fi, :],
                            rhs=G[fi],
                            start=(fi == 0),
                            stop=(fi == fb_bf_ - 1),
                        )
                    ps8 = ps8p.tile([P, mb_], f32, tag="ps8")
                    for u in range(p8_):
                        nc.tensor.matmul(
                            ps8,
                            lhsT=w28_t[:, u],
                            rhs=g8_t[:, 2 * u : 2 * u + 2, :],
                            start=(u == 0),
                            stop=(u == p8_ - 1),
                            perf_mode=DR,
                        )
                    # oT = ps2 + (ps8 * 2^-10 + b2)
                    t8 = t8p.tile([P, mb_], f32, tag="t8")
                    nc.scalar.activation(
                        t8, ps8, IDENT,
                        bias=b2_sb[:, db : db + 1],
                        scale=1.0 / W8SCALE,
                    )
                    o_t = outp.tile([P, mb_], f32, tag="o")
                    nc.vector.tensor_add(o_t, ps2, t8)
                    nc.sync.dma_start(ot_r[:, db, m0 : m0 + mb_], o_t)

    nc.compile()
    return nc


_NC = None


def _get_nc():
    global _NC
    if _NC is None:
        _NC = _build_nc()
    return _NC


def _shard_inputs(x, w1, b1, w2, b2):
    x = np.asarray(x)
    w1, b1 = np.asarray(w1), np.asarray(b1)
    w2, b2 = np.asarray(w2), np.asarray(b2)
    in_maps = []
    for e in range(E):
        xeT = x[:, e * CAP : (e + 1) * CAP, :].reshape(M, D).T.astype(BF16)
        in_maps.append(
            {
                "xt": _pack_xt(xeT, N_MB, MB, KB),
                "w1": _pack_w1(w1[e].astype(BF16), N_FS, FS * P, KB),
                "w2": _pack_w2_bf(w2[e].astype(BF16), DB, FB_BF),
                "w28": _pack_w2_fp8(w2[e].astype(np.float32), DB, FB_BF, P8),
                "b1": np.ascontiguousarray(
                    b1[e].reshape(FB, P).T.astype(np.float32)
                ),
                "b2": np.ascontiguousarray(
                    b2[e].reshape(DB, P).T.astype(np.float32)
                ),
            }
        )
    return in_maps


def _ensure_axon_hooks():
    """bass_utils imports antenv.axon_hooks when tracing is requested (e.g.
    via BASS_TRACE); this image's antenv lacks that module. Provide it, and
    register the local NTFF hook when available, so tracing degrades
    gracefully instead of raising ImportError."""
    import types

    try:
        import antenv
    except Exception:
        return
    if getattr(antenv, "axon_hooks", None) is not None:
        return
    try:
        import antenv.axon_hooks  # noqa: F401

        return
    except Exception:
        pass
    mod = types.ModuleType("antenv.axon_hooks")
    _h = [None]
    mod.get_axon_ntff_profile_hook = lambda: _h[0]
    mod.set_axon_ntff_profile_hook = lambda h: _h.__setitem__(0, h)
    sys.modules["antenv.axon_hooks"] = mod
    antenv.axon_hooks = mod
    try:
        from trn_agent_boot.trn_boot import _ntff_profile_via_ctypes

        hk = _ntff_profile_via_ctypes("/opt/axon/libaxon_pjrt.so")
        if hk is not None:
            mod.set_axon_ntff_profile_hook(hk)
    except Exception:
        pass
    try:
        # artifact upload needs S3; fall back to the local dir on failure
        import concourse.bass_utils as bu

        _orig_upload = bu.upload_artifacts

        def _safe_upload(tmpdir):
            try:
                return _orig_upload(tmpdir)
            except Exception:
                return "local://" + str(tmpdir)

        bu.upload_artifacts = _safe_upload
    except Exception:
        pass


def _run_with_retry(nc, in_maps, trace, attempts=3):
    """The device intermittently fails with NRT_EXEC_UNIT_UNRECOVERABLE even
    on known-good kernels; retry after resetting the jax backend."""
    import time

    from concourse.bass_utils import run_bass_kernel_spmd

    last = None
    for attempt in range(attempts):
        try:
            return run_bass_kernel_spmd(
                nc,
                in_maps,
                core_ids=list(range(E)),
                trace=trace and attempt == 0,
            )
        except Exception as e:  # noqa: BLE001
            last = e
            if attempt == attempts - 1:
                break
            time.sleep(45 * (attempt + 1))
            try:
                import jax.extend.backend

                jax.extend.backend.clear_backends()
            except Exception:
                pass
    raise last


def kernel(x, w1, b1, w2, b2):
    global LAST_EXEC_NS, LAST_RESULTS
    _ensure_axon_hooks()

    nc = _get_nc()
    in_maps = _shard_inputs(x, w1, b1, w2, b2)
    res = _run_with_retry(nc, in_maps, TRACE)
    LAST_EXEC_NS = res.exec_time_ns
    LAST_RESULTS = res
    out = np.empty((B, E * CAP, D), dtype=np.float32)
    for e in range(E):
        out[:, e * CAP : (e + 1) * CAP, :] = (
            res.results[e]["ot"].T.reshape(B, CAP, D)
        )
    return out
